# revision 12
# baseline (speedup 1.0000x reference)
"""BiLSTM (B=64, L=256, D=512, H=512) on 8 Trainium2 NeuronCores.

Sharding: 8 cores = 2 directions x 4 batch-slices of 16 (weights replicated
per direction, sequential time loop local to each core).  Backward-direction
cores receive time-reversed x, so every core runs the identical SPMD program.

v4 design:
  - all matmuls bf16 (1 cycle/moving-row, single pass).
  - gate layout: TWO psum banks per step, Pfo=[f,o] and Pgi=[g,i], each
    [128, 256] with gate blocks of 128 cols; block = [4 h-chunks x
    32-partition groups, 128 free] (partition = 32*hc + b, free = h%128).
    Two banks -> the ACT tail for [f,o] starts while [g,i] matmuls run.
  - x-part GEMM is FOLDED into the recurrence loop (one (m,n) chunklet
    every 2 steps + 16-chunklet prologue): keeps the PE busy enough that
    the HAM clock gate stays at 2.4 GHz (v2 ran at 1.2 GHz 88% of the
    time), and hides the whole phase-1 cost in tail-wait PE idle slots.
  - transpose factoring: hT = T(sigma_o) * T(tanh c) elementwise; the two
    PE transposes run inside the tail and ONE DVE mul yields the bf16
    stationary for the next step directly.  out_h is written in the
    transposed layout [hl, 32k+b]; the host unscrambles.
  - per-step PE order: h-mm(gi) | h-mm(fo) | inject(t+1) | chunklet |
    T_o | T_C  -- inject/chunklet fill the ACT/DVE tail wait.
  - all-sigmoid tail via tanh(x) = 2*sigmoid(2x)-1 with power-of-2 weight
    folding (exact in bf16): gate g's weights x2 (block holds 2g), cell
    state kept as C = 2c, hidden state kept as h/2 (Wh x2 compensates;
    host doubles the output).  One merged sigmoid per psum bank + one
    sigmoid(C): 3 ACT ops/step instead of 5, all the same function.
"""

import numpy as np
import ml_dtypes

from concourse import tile, mybir, bacc
from concourse.bass_utils import run_bass_kernel_spmd
from concourse.masks import make_identity
from concourse.alu_op_type import AluOpType

FP = mybir.dt.float32
BF = mybir.dt.bfloat16
AF = mybir.ActivationFunctionType

B = 16        # local batch per core
L = 256       # timesteps
D = 512       # input dim
H = 512       # hidden
NG = 4 * H    # gate width
TOK = L * B   # tokens per core
NM = TOK // 128  # 32 x-part token tiles

# gate order on device: blocks [g, i, f, o]; banks: gi = blocks 0:2,
# fo = blocks 2:4
GATES = "gifo"

N_PRO = 16    # prologue chunklets (4 m-tiles)
XPR_BUFS = 6  # xp ring depth in m-tiles

_CACHED_NC = None


def _build():
    nc = bacc.Bacc("TRN2", target_bir_lowering=False, debug=False)

    xT = nc.dram_tensor("xT", [D, TOK], BF, kind="ExternalInput").ap()
    Wx = nc.dram_tensor("Wx", [D, NG], BF, kind="ExternalInput").ap()
    Wh = nc.dram_tensor("Wh", [H, NG], BF, kind="ExternalInput").ap()
    bias = nc.dram_tensor("bias", [1, NG], BF, kind="ExternalInput").ap()
    out_h = nc.dram_tensor("out_h", [L, 128, 112], BF, kind="ExternalOutput").ap()

    with tile.TileContext(nc, trace_sim=False) as tc:
        with tc.tile_pool(name="wpool", bufs=1) as wpool, \
             tc.tile_pool(name="cpool", bufs=1) as cpool, \
             tc.tile_pool(name="xpr", bufs=XPR_BUFS) as xpr, \
             tc.tile_pool(name="p1x", bufs=3) as p1x, \
             tc.tile_pool(name="p1ps", bufs=2, space="PSUM") as p1ps, \
             tc.tile_pool(name="st", bufs=2) as st, \
             tc.tile_pool(name="ch", bufs=2) as ch, \
             tc.tile_pool(name="gps_fo", bufs=2, space="PSUM") as gps_fo, \
             tc.tile_pool(name="gps_gi", bufs=2, space="PSUM") as gps_gi, \
             tc.tile_pool(name="tps", bufs=1, space="PSUM") as tps:

            # ---- persistent weights / identity ----
            Wx_t, Wh_t = [], []
            for k in range(4):
                wt = wpool.tile([128, NG], BF, tag=f"wx{k}", name=f"wx{k}")
                nc.sync.dma_start(wt[:], Wx[128 * k:128 * (k + 1), :])
                Wx_t.append(wt)
            for k in range(4):
                wt = wpool.tile([128, NG], BF, tag=f"wh{k}", name=f"wh{k}")
                nc.sync.dma_start(wt[:], Wh[128 * k:128 * (k + 1), :])
                Wh_t.append(wt)
            bias_t = wpool.tile([1, NG], BF)
            nc.sync.dma_start(bias_t[:], bias[:, :])
            ones_t = cpool.tile([1, 128], BF)
            nc.vector.memset(ones_t[:, :], 1.0)
            ident = cpool.tile([128, 128], BF)
            make_identity(nc, ident[:, :])

            # ---- gate psum ring buffers, kept by step parity ----
            P_fo, P_gi = [None, None], [None, None]
            for z in range(2):
                P_fo[z] = gps_fo.tile([128, 256], FP, tag="Pfo", name=f"Pfo{z}")
                nc.vector.memset(P_fo[z][:, :], 0.0)
                P_gi[z] = gps_gi.tile([128, 256], FP, tag="Pgi", name=f"Pgi{z}")
                nc.vector.memset(P_gi[z][:, :], 0.0)

            # ---- x-part chunklet machinery (folded phase 1) ----
            xp_tiles = {}       # m -> 4D tile [128 tok, 4 G, 4 hc, 128]
            state = {"c": 0, "xm": None}

            def emit_chunklet():
                c = state["c"]
                if c >= 4 * NM:
                    return
                state["c"] += 1
                m, n = divmod(c, 4)
                if n == 0:
                    xm = p1x.tile([128, 4, 128], BF, tag="xm", name="xm")
                    for k in range(4):
                        nc.sync.dma_start(
                            xm[:, k, :],
                            xT[128 * k:128 * (k + 1), 128 * m:128 * (m + 1)])
                    xp_tiles[m] = xpr.tile(
                        [128, 4, 4, 128], BF, tag="xpr", name=f"xp{m}")
                    state["xm"] = xm
                xm = state["xm"]
                ps = p1ps.tile([128, 512], FP, tag="ps1", name="ps1")
                for k in range(4):
                    nc.tensor.matmul(
                        ps[:, :], xm[:, k, :],
                        Wx_t[k][:, 512 * n:512 * (n + 1)],
                        start=(k == 0), stop=False)
                nc.tensor.matmul(
                    ps[:, :], ones_t[:, :], bias_t[:, 512 * n:512 * (n + 1)],
                    start=False, stop=True)
                dst = xp_tiles[m][:, n, :, :]
                if c % 2 == 0:
                    nc.scalar.copy(dst, ps[:, :])
                else:
                    nc.vector.tensor_copy(dst, ps[:, :])

            def emit_inject(t):
                """x-part injection for step t.  start=True clears the whole
                32-partition column group's has_written bits, so exactly ONE
                start mm per (bank, column-group)."""
                m, r = divmod(t, 8)
                w0 = 32 * (r // 2)
                so = B * (r % 2)
                xp4 = xp_tiles[m]
                for gl, Pb in ((0, P_gi[t % 2]), (2, P_fo[t % 2])):
                    for hc in range(4):
                        nc.tensor.matmul(
                            Pb[32 * hc:32 * hc + B, 0:256],
                            ident[w0:w0 + 32, w0 + so:w0 + so + B],
                            xp4[w0:w0 + 32, gl:gl + 2, hc, :],
                            start=True, stop=False,
                            tile_position=(w0, 32 * hc))

            # ---- zero-init state ----
            c_prev = st.tile([112, 128], FP, tag="c", name="c0")
            nc.vector.memset(c_prev[:, :], 0.0)
            hTs_prev = st.tile([128, 112], BF, tag="hTs", name="hTs0")
            nc.vector.memset(hTs_prev[:, :], 0.0)

            # ---- prologue: first chunklets + inject(0) ----
            for _ in range(N_PRO):
                emit_chunklet()
            emit_inject(0)

            for t in range(L):
                Pfo = P_fo[t % 2]
                Pgi = P_gi[t % 2]

                # h-part matmuls: bank gi first (its tail chain is longest)
                for gl, Pb in ((0, Pgi), (2, Pfo)):
                    for k in range(4):
                        for Gb in range(2):
                            for hc in range(4):
                                nc.tensor.matmul(
                                    Pb[32 * hc:32 * hc + B,
                                       128 * Gb:128 * (Gb + 1)],
                                    hTs_prev[:, 32 * k:32 * k + B],
                                    Wh_t[k][:, ((gl + Gb) * 4 + hc) * 128:
                                            ((gl + Gb) * 4 + hc + 1) * 128],
                                    start=False, stop=(k == 3),
                                    tile_position=(0, 32 * hc))

                # PE filler work during this step's ACT/DVE tail
                if t + 1 < L:
                    emit_inject(t + 1)
                if t % 2 == 0:
                    emit_chunklet()

                # ---- tail (all-sigmoid; see header) ----
                # bank gi: one merged sigmoid over [2g|i]
                s_gi = ch.tile([112, 256], BF, tag="sgi", name="sgi")
                nc.scalar.activation(s_gi[:, :], Pgi[0:112, :], AF.Sigmoid)
                # t2h = (sig(2g) - 0.5) * sig(i)  ==  tanh(g)*sig(i)/2
                t2h = ch.tile([112, 128], FP, tag="t2h", name="t2h")
                nc.vector.scalar_tensor_tensor(
                    t2h[:, :], s_gi[:, 0:128], 0.5, s_gi[:, 128:256],
                    AluOpType.subtract, AluOpType.mult)
                # bank fo: one merged sigmoid over [f|o]
                s_fo = ch.tile([112, 256], BF, tag="sfo", name="sfo")
                nc.scalar.activation(s_fo[:, :], Pfo[0:112, :], AF.Sigmoid)
                t1 = ch.tile([112, 128], FP, tag="t1", name="t1")
                nc.vector.tensor_mul(t1[:, :], s_fo[:, 0:128], c_prev[:, :])
                # C_new = 4*t2h + t1   (state C = 2c)
                c_new = st.tile([112, 128], FP, tag="c", name="c")
                nc.vector.scalar_tensor_tensor(
                    c_new[:, :], t2h[:, :], 4.0, t1[:, :],
                    AluOpType.mult, AluOpType.add)
                # sig(C): tanh(c) = 2*sig(C) - 1
                sC = ch.tile([112, 128], BF, tag="sC", name="sC")
                nc.scalar.activation(sC[:, :], c_new[:, :], AF.Sigmoid)

                # transposes (PE) + one fused DVE op -> h/2 transposed.
                # DVE reads at most one PSUM operand, so T(sigma_o) is staged
                # to SBUF early (off the critical chain).
                TT = tps.tile([128, 224], BF, tag="TT", name="TT")
                nc.tensor.transpose(
                    TT[:, 0:112], s_fo[0:112, 128:256], ident[0:112, 0:112])
                To_sb = ch.tile([128, 112], BF, tag="To", name="To")
                nc.vector.tensor_copy(To_sb[:, :], TT[:, 0:112])
                nc.tensor.transpose(
                    TT[:, 112:224], sC[0:112, :], ident[0:112, 0:112])
                # h/2 = (sig(C)^T - 0.5) * sig(o)^T
                hTs_new = st.tile([128, 112], BF, tag="hTs", name="hTs")
                nc.vector.scalar_tensor_tensor(
                    hTs_new[:, :], TT[:, 112:224], 0.5, To_sb[:, :],
                    AluOpType.subtract, AluOpType.mult)

                nc.sync.dma_start(out_h[t, :, :], hTs_new[:, :])

                c_prev = c_new
                hTs_prev = hTs_new
    nc.compile()
    return nc


def _host_prepare(x_full, weights, direction, bslice):
    xs = x_full[bslice]
    if direction == "bw":
        xs = xs[:, ::-1, :]
    xT = np.ascontiguousarray(xs.transpose(2, 1, 0).reshape(D, TOK))
    Wc = np.concatenate(
        [np.asarray(weights[f"W_{direction}_{n}"]).T for n in GATES], axis=1)
    bc = np.concatenate(
        [np.asarray(weights[f"b_{direction}_{n}"]) for n in GATES])[None, :]
    # tanh-as-sigmoid folding: gate g cols x2 (block holds 2g); h-part rows
    # x2 (recurrent state is h/2); both exact power-of-2 scalings in bf16.
    Wc = Wc.copy(); bc = bc.copy()
    Wc[:, 0:H] *= 2.0
    bc[:, 0:H] *= 2.0
    Wc[D:, :] *= 2.0
    bf = ml_dtypes.bfloat16
    return {"xT": xT.astype(bf),
            "Wx": np.ascontiguousarray(Wc[:D]).astype(bf),
            "Wh": np.ascontiguousarray(Wc[D:]).astype(bf),
            "bias": np.ascontiguousarray(bc).astype(bf)}


def prepare(inputs):
    """Build (cached) the bass program and the 8 per-core input maps."""
    global _CACHED_NC
    inputs = {k: np.asarray(v) for k, v in inputs.items()}
    x = inputs["x"]
    Bx, Lx, _ = x.shape
    assert (Bx, Lx) == (64, L)

    if _CACHED_NC is None:
        _CACHED_NC = _build()
    nc = _CACHED_NC

    in_maps = []
    for ci in range(8):
        d = "fw" if ci < 4 else "bw"
        bs = (ci % 4) * B
        in_maps.append(_host_prepare(x, inputs, d, slice(bs, bs + B)))
    return nc, in_maps


def _unshard_core(oh):
    """out_h [L, 128, 112] bf16 (transposed h) -> [L, 16, 512] f32.
    h[t, b, 128*k + hl] = oh[t, hl, 32*k + b]."""
    a = np.asarray(oh).astype(np.float32) * 2.0  # device stores h/2
    parts = [a[:, :, 32 * k:32 * k + B].transpose(0, 2, 1) for k in range(4)]
    return np.concatenate(parts, axis=2)  # [L, B, 512]


def kernel(**inputs):
    inputs = {k: np.asarray(v) for k, v in inputs.items()}
    x = inputs["x"]
    Bx = x.shape[0]
    nc, in_maps = prepare(inputs)
    meta = [("fw" if ci < 4 else "bw", (ci % 4) * B) for ci in range(8)]

    res = run_bass_kernel_spmd(nc, in_maps, core_ids=list(range(8)))

    hf = np.zeros((L, Bx, H), np.float32)
    hb = np.zeros((L, Bx, H), np.float32)
    for ci in range(8):
        d, bs = meta[ci]
        oh = _unshard_core(res.results[ci]["out_h"])  # (L, 16, H) time-major
        if d == "fw":
            hf[:, bs:bs + B, :] = oh
        else:
            hb[:, bs:bs + B, :] = oh[::-1]

    # faithful to the reference: stack time-major, flatten, hstack, reshape
    flat = np.concatenate([hf.reshape(-1, H), hb.reshape(-1, H)], axis=1)
    return flat.reshape(Bx, L, 2 * H).astype(np.float32)


# revision 13
# speedup vs baseline: 1.0159x; 1.0159x over previous
"""BiLSTM (B=64, L=256, D=512, H=512) on 8 Trainium2 NeuronCores.

Sharding: 8 cores = 2 directions x 4 batch-slices of 16 (weights replicated
per direction, sequential time loop local to each core).  Backward-direction
cores receive time-reversed x, so every core runs the identical SPMD program.

v4 design:
  - all matmuls bf16 (1 cycle/moving-row, single pass).
  - gate layout: TWO psum banks per step, Pfo=[f,o] and Pgi=[g,i], each
    [128, 256] with gate blocks of 128 cols; block = [4 h-chunks x
    32-partition groups, 128 free] (partition = 32*hc + b, free = h%128).
    Two banks -> the ACT tail for [f,o] starts while [g,i] matmuls run.
  - x-part GEMM is FOLDED into the recurrence loop (one (m,n) chunklet
    every 2 steps + 16-chunklet prologue): keeps the PE busy enough that
    the HAM clock gate stays at 2.4 GHz (v2 ran at 1.2 GHz 88% of the
    time), and hides the whole phase-1 cost in tail-wait PE idle slots.
  - transpose factoring: hT = T(sigma_o) * T(tanh c) elementwise; the two
    PE transposes run inside the tail and ONE DVE mul yields the bf16
    stationary for the next step directly.  out_h is written in the
    transposed layout [hl, 32k+b]; the host unscrambles.
  - per-step PE order: h-mm(gi) | h-mm(fo) | inject(t+1) | chunklet |
    T_o | T_C  -- inject/chunklet fill the ACT/DVE tail wait.
  - merged-activation tail: gate g's weights are doubled (exact in bf16)
    so its block holds 2g, and tanh(g) = 2*sigmoid(2g)-1 lets ONE sigmoid
    cover the whole [2g|i] bank (s_gi kept fp32 to avoid the sigma-0.5
    cancellation); fused scalar_tensor_tensor ops recover the cell
    update.  3 ACT ops/step (sigma_gi, sigma_fo, tanh_c) instead of 5.
"""

import numpy as np
import ml_dtypes

from concourse import tile, mybir, bacc
from concourse.bass_utils import run_bass_kernel_spmd
from concourse.masks import make_identity
from concourse.alu_op_type import AluOpType

FP = mybir.dt.float32
BF = mybir.dt.bfloat16
AF = mybir.ActivationFunctionType

B = 16        # local batch per core
L = 256       # timesteps
D = 512       # input dim
H = 512       # hidden
NG = 4 * H    # gate width
TOK = L * B   # tokens per core
NM = TOK // 128  # 32 x-part token tiles

# gate order on device: blocks [g, i, f, o]; banks: gi = blocks 0:2,
# fo = blocks 2:4
GATES = "gifo"

N_PRO = 16    # prologue chunklets (4 m-tiles)
XPR_BUFS = 6  # xp ring depth in m-tiles

_CACHED_NC = None


def _build():
    nc = bacc.Bacc("TRN2", target_bir_lowering=False, debug=False)

    xT = nc.dram_tensor("xT", [D, TOK], BF, kind="ExternalInput").ap()
    Wx = nc.dram_tensor("Wx", [D, NG], BF, kind="ExternalInput").ap()
    Wh = nc.dram_tensor("Wh", [H, NG], BF, kind="ExternalInput").ap()
    bias = nc.dram_tensor("bias", [1, NG], BF, kind="ExternalInput").ap()
    out_h = nc.dram_tensor("out_h", [L, 128, 112], BF, kind="ExternalOutput").ap()

    with tile.TileContext(nc, trace_sim=False) as tc:
        with tc.tile_pool(name="wpool", bufs=1) as wpool, \
             tc.tile_pool(name="cpool", bufs=1) as cpool, \
             tc.tile_pool(name="xpr", bufs=XPR_BUFS) as xpr, \
             tc.tile_pool(name="p1x", bufs=3) as p1x, \
             tc.tile_pool(name="p1ps", bufs=2, space="PSUM") as p1ps, \
             tc.tile_pool(name="st", bufs=2) as st, \
             tc.tile_pool(name="ch", bufs=2) as ch, \
             tc.tile_pool(name="gps_fo", bufs=2, space="PSUM") as gps_fo, \
             tc.tile_pool(name="gps_gi", bufs=2, space="PSUM") as gps_gi, \
             tc.tile_pool(name="tps", bufs=1, space="PSUM") as tps:

            # ---- persistent weights / identity ----
            Wx_t, Wh_t = [], []
            for k in range(4):
                wt = wpool.tile([128, NG], BF, tag=f"wx{k}", name=f"wx{k}")
                nc.sync.dma_start(wt[:], Wx[128 * k:128 * (k + 1), :])
                Wx_t.append(wt)
            for k in range(4):
                wt = wpool.tile([128, NG], BF, tag=f"wh{k}", name=f"wh{k}")
                nc.sync.dma_start(wt[:], Wh[128 * k:128 * (k + 1), :])
                Wh_t.append(wt)
            bias_t = wpool.tile([1, NG], BF)
            nc.sync.dma_start(bias_t[:], bias[:, :])
            ones_t = cpool.tile([1, 128], BF)
            nc.vector.memset(ones_t[:, :], 1.0)
            ident = cpool.tile([128, 128], BF)
            make_identity(nc, ident[:, :])

            # ---- gate psum ring buffers, kept by step parity ----
            P_fo, P_gi = [None, None], [None, None]
            for z in range(2):
                P_fo[z] = gps_fo.tile([128, 256], FP, tag="Pfo", name=f"Pfo{z}")
                nc.vector.memset(P_fo[z][:, :], 0.0)
                P_gi[z] = gps_gi.tile([128, 256], FP, tag="Pgi", name=f"Pgi{z}")
                nc.vector.memset(P_gi[z][:, :], 0.0)

            # ---- x-part chunklet machinery (folded phase 1) ----
            xp_tiles = {}       # m -> 4D tile [128 tok, 4 G, 4 hc, 128]
            state = {"c": 0, "xm": None}

            def emit_chunklet():
                c = state["c"]
                if c >= 4 * NM:
                    return
                state["c"] += 1
                m, n = divmod(c, 4)
                if n == 0:
                    xm = p1x.tile([128, 4, 128], BF, tag="xm", name="xm")
                    for k in range(4):
                        nc.sync.dma_start(
                            xm[:, k, :],
                            xT[128 * k:128 * (k + 1), 128 * m:128 * (m + 1)])
                    xp_tiles[m] = xpr.tile(
                        [128, 4, 4, 128], BF, tag="xpr", name=f"xp{m}")
                    state["xm"] = xm
                xm = state["xm"]
                ps = p1ps.tile([128, 512], FP, tag="ps1", name="ps1")
                for k in range(4):
                    nc.tensor.matmul(
                        ps[:, :], xm[:, k, :],
                        Wx_t[k][:, 512 * n:512 * (n + 1)],
                        start=(k == 0), stop=False)
                nc.tensor.matmul(
                    ps[:, :], ones_t[:, :], bias_t[:, 512 * n:512 * (n + 1)],
                    start=False, stop=True)
                dst = xp_tiles[m][:, n, :, :]
                if c % 2 == 0:
                    nc.scalar.copy(dst, ps[:, :])
                else:
                    nc.vector.tensor_copy(dst, ps[:, :])

            def emit_inject(t):
                """x-part injection for step t.  start=True clears the whole
                32-partition column group's has_written bits, so exactly ONE
                start mm per (bank, column-group)."""
                m, r = divmod(t, 8)
                w0 = 32 * (r // 2)
                so = B * (r % 2)
                xp4 = xp_tiles[m]
                for gl, Pb in ((0, P_gi[t % 2]), (2, P_fo[t % 2])):
                    for hc in range(4):
                        nc.tensor.matmul(
                            Pb[32 * hc:32 * hc + B, 0:256],
                            ident[w0:w0 + 32, w0 + so:w0 + so + B],
                            xp4[w0:w0 + 32, gl:gl + 2, hc, :],
                            start=True, stop=False,
                            tile_position=(w0, 32 * hc))

            # ---- zero-init state ----
            c_prev = st.tile([112, 128], FP, tag="c", name="c0")
            nc.vector.memset(c_prev[:, :], 0.0)
            hTs_prev = st.tile([128, 112], BF, tag="hTs", name="hTs0")
            nc.vector.memset(hTs_prev[:, :], 0.0)

            # ---- prologue: first chunklets + inject(0) ----
            for _ in range(N_PRO):
                emit_chunklet()
            emit_inject(0)

            for t in range(L):
                Pfo = P_fo[t % 2]
                Pgi = P_gi[t % 2]

                # h-part matmuls: bank gi first (its tail chain is longest)
                for gl, Pb in ((0, Pgi), (2, Pfo)):
                    for k in range(4):
                        for Gb in range(2):
                            for hc in range(4):
                                nc.tensor.matmul(
                                    Pb[32 * hc:32 * hc + B,
                                       128 * Gb:128 * (Gb + 1)],
                                    hTs_prev[:, 32 * k:32 * k + B],
                                    Wh_t[k][:, ((gl + Gb) * 4 + hc) * 128:
                                            ((gl + Gb) * 4 + hc + 1) * 128],
                                    start=False, stop=(k == 3),
                                    tile_position=(0, 32 * hc))

                # PE filler work during this step's ACT/DVE tail
                if t + 1 < L:
                    emit_inject(t + 1)
                if t % 2 == 0:
                    emit_chunklet()

                # ---- tail (all-sigmoid; see header) ----
                # bank gi: one merged sigmoid over [2g|i]; fp32 so the
                # (sigma(2g) - 0.5) subtraction does not cancel in bf16
                s_gi = ch.tile([112, 256], FP, tag="sgi", name="sgi")
                nc.scalar.activation(s_gi[:, :], Pgi[0:112, :], AF.Sigmoid)
                # t2h = (sig(2g) - 0.5) * sig(i)  ==  tanh(g)*sig(i)/2
                t2h = ch.tile([112, 128], FP, tag="t2h", name="t2h")
                nc.vector.scalar_tensor_tensor(
                    t2h[:, :], s_gi[:, 0:128], 0.5, s_gi[:, 128:256],
                    AluOpType.subtract, AluOpType.mult)
                # bank fo: one merged sigmoid over [f|o]
                s_fo = ch.tile([112, 256], BF, tag="sfo", name="sfo")
                nc.scalar.activation(s_fo[:, :], Pfo[0:112, :], AF.Sigmoid)
                t1 = ch.tile([112, 128], FP, tag="t1", name="t1")
                nc.vector.tensor_mul(t1[:, :], s_fo[:, 0:128], c_prev[:, :])
                # c_new = 2*t2h + t1   (t2h = tanh(g)*sig(i)/2)
                c_new = st.tile([112, 128], FP, tag="c", name="c")
                nc.vector.scalar_tensor_tensor(
                    c_new[:, :], t2h[:, :], 2.0, t1[:, :],
                    AluOpType.mult, AluOpType.add)
                th = ch.tile([112, 128], BF, tag="th", name="th")
                nc.scalar.activation(th[:, :], c_new[:, :], AF.Tanh)

                # transposes (PE) + one fused DVE op -> h/2 transposed.
                # DVE reads at most one PSUM operand, so T(sigma_o) is staged
                # to SBUF early (off the critical chain).
                TT = tps.tile([128, 224], BF, tag="TT", name="TT")
                nc.tensor.transpose(
                    TT[:, 0:112], s_fo[0:112, 128:256], ident[0:112, 0:112])
                To_sb = ch.tile([128, 112], BF, tag="To", name="To")
                nc.vector.tensor_copy(To_sb[:, :], TT[:, 0:112])
                nc.tensor.transpose(
                    TT[:, 112:224], th[0:112, :], ident[0:112, 0:112])
                # h = tanh(c)^T * sig(o)^T
                hTs_new = st.tile([128, 112], BF, tag="hTs", name="hTs")
                nc.vector.tensor_mul(
                    hTs_new[:, :], TT[:, 112:224], To_sb[:, :])

                nc.sync.dma_start(out_h[t, :, :], hTs_new[:, :])

                c_prev = c_new
                hTs_prev = hTs_new
    nc.compile()
    return nc


def _host_prepare(x_full, weights, direction, bslice):
    xs = x_full[bslice]
    if direction == "bw":
        xs = xs[:, ::-1, :]
    xT = np.ascontiguousarray(xs.transpose(2, 1, 0).reshape(D, TOK))
    Wc = np.concatenate(
        [np.asarray(weights[f"W_{direction}_{n}"]).T for n in GATES], axis=1)
    bc = np.concatenate(
        [np.asarray(weights[f"b_{direction}_{n}"]) for n in GATES])[None, :]
    # tanh-as-sigmoid folding: gate g cols x2 (block holds 2g) -- exact
    # power-of-2 scaling in bf16.
    Wc = Wc.copy(); bc = bc.copy()
    Wc[:, 0:H] *= 2.0
    bc[:, 0:H] *= 2.0
    bf = ml_dtypes.bfloat16
    return {"xT": xT.astype(bf),
            "Wx": np.ascontiguousarray(Wc[:D]).astype(bf),
            "Wh": np.ascontiguousarray(Wc[D:]).astype(bf),
            "bias": np.ascontiguousarray(bc).astype(bf)}


def prepare(inputs):
    """Build (cached) the bass program and the 8 per-core input maps."""
    global _CACHED_NC
    inputs = {k: np.asarray(v) for k, v in inputs.items()}
    x = inputs["x"]
    Bx, Lx, _ = x.shape
    assert (Bx, Lx) == (64, L)

    if _CACHED_NC is None:
        _CACHED_NC = _build()
    nc = _CACHED_NC

    in_maps = []
    for ci in range(8):
        d = "fw" if ci < 4 else "bw"
        bs = (ci % 4) * B
        in_maps.append(_host_prepare(x, inputs, d, slice(bs, bs + B)))
    return nc, in_maps


def _unshard_core(oh):
    """out_h [L, 128, 112] bf16 (transposed h) -> [L, 16, 512] f32.
    h[t, b, 128*k + hl] = oh[t, hl, 32*k + b]."""
    a = np.asarray(oh).astype(np.float32)
    parts = [a[:, :, 32 * k:32 * k + B].transpose(0, 2, 1) for k in range(4)]
    return np.concatenate(parts, axis=2)  # [L, B, 512]


def kernel(**inputs):
    inputs = {k: np.asarray(v) for k, v in inputs.items()}
    x = inputs["x"]
    Bx = x.shape[0]
    nc, in_maps = prepare(inputs)
    meta = [("fw" if ci < 4 else "bw", (ci % 4) * B) for ci in range(8)]

    res = run_bass_kernel_spmd(nc, in_maps, core_ids=list(range(8)))

    hf = np.zeros((L, Bx, H), np.float32)
    hb = np.zeros((L, Bx, H), np.float32)
    for ci in range(8):
        d, bs = meta[ci]
        oh = _unshard_core(res.results[ci]["out_h"])  # (L, 16, H) time-major
        if d == "fw":
            hf[:, bs:bs + B, :] = oh
        else:
            hb[:, bs:bs + B, :] = oh[::-1]

    # faithful to the reference: stack time-major, flatten, hstack, reshape
    flat = np.concatenate([hf.reshape(-1, H), hb.reshape(-1, H)], axis=1)
    return flat.reshape(Bx, L, 2 * H).astype(np.float32)


# revision 15
# speedup vs baseline: 1.0504x; 1.0339x over previous
"""BiLSTM (B=64, L=256, D=512, H=512) on 8 Trainium2 NeuronCores.

Sharding: 8 cores = 2 directions x 4 batch-slices of 16 (weights replicated
per direction, sequential time loop local to each core).  Backward-direction
cores receive time-reversed x, so every core runs the identical SPMD program.

v6 design:
  - all matmuls bf16 (1 cycle/moving-row, single pass).
  - gate layout: TWO psum banks per step, Pfo=[f,o] and Pgi=[g,i], each
    [128, 256] with gate blocks of 128 cols; block = [4 h-chunks x
    32-partition groups, 128 free] (partition = 32*hc + b, free = h%128).
    Two banks -> the ACT tail for [f,o] starts while [g,i] matmuls run.
  - x-part GEMM is FOLDED into the recurrence loop (one (m,n) chunklet
    every 2 steps + 16-chunklet prologue): keeps the PE busy enough that
    the HAM clock gate stays at 2.4 GHz (v2 ran at 1.2 GHz 88% of the
    time), and hides the whole phase-1 cost in tail-wait PE idle slots.
  - transpose factoring: hT = T(sigma_o) * T(tanh c) elementwise; the two
    PE transposes run inside the tail and ONE DVE mul yields the bf16
    stationary for the next step directly.  out_h is written in the
    transposed layout [hl, 32k+b]; the host unscrambles.
  - the Tile scheduler is a per-engine ready-heap ordered by emission
    priority: all filler PE work (inject(t+1), x-part chunklets, HAM
    warm-keeper dummies) is emitted at LOW priority so the tail's
    semaphore thresholds never include it, and it soaks up PE idle.
  - low-priority dummy matmuls keep the PE busy so the HAM clock gate
    ramps to 2.4 GHz and stays there (otherwise every matmul runs at
    1.2 GHz: bursts are too short for the 3.4 us activity window).
  - merged-activation tail: gate g's weights are doubled (exact in bf16)
    so its block holds 2g, and tanh(g) = 2*sigmoid(2g)-1 lets ONE sigmoid
    cover the whole [2g|i] bank (s_gi kept fp32 to avoid the sigma-0.5
    cancellation); fused scalar_tensor_tensor ops recover the cell
    update.  3 ACT ops/step (sigma_gi, sigma_fo, tanh_c) instead of 5.
"""

import numpy as np
import ml_dtypes

from concourse import tile, mybir, bacc
from concourse.bass_utils import run_bass_kernel_spmd
from concourse.masks import make_identity
from concourse.alu_op_type import AluOpType

FP = mybir.dt.float32
BF = mybir.dt.bfloat16
AF = mybir.ActivationFunctionType

B = 16        # local batch per core
L = 256       # timesteps
D = 512       # input dim
H = 512       # hidden
NG = 4 * H    # gate width
TOK = L * B   # tokens per core
NM = TOK // 128  # 32 x-part token tiles

# gate order on device: blocks [g, i, f, o]; banks: gi = blocks 0:2,
# fo = blocks 2:4
GATES = "gifo"

N_PRO = 16    # prologue chunklets (4 m-tiles)
XPR_BUFS = 6  # xp ring depth in m-tiles

_CACHED_NC = None


def _build():
    nc = bacc.Bacc("TRN2", target_bir_lowering=False, debug=False)

    xT = nc.dram_tensor("xT", [D, TOK], BF, kind="ExternalInput").ap()
    Wx = nc.dram_tensor("Wx", [D, NG], BF, kind="ExternalInput").ap()
    Wh = nc.dram_tensor("Wh", [H, NG], BF, kind="ExternalInput").ap()
    bias = nc.dram_tensor("bias", [1, NG], BF, kind="ExternalInput").ap()
    out_h = nc.dram_tensor("out_h", [L, 128, 112], BF, kind="ExternalOutput").ap()

    with tile.TileContext(nc, trace_sim=False) as tc:
        with tc.tile_pool(name="wpool", bufs=1) as wpool, \
             tc.tile_pool(name="cpool", bufs=1) as cpool, \
             tc.tile_pool(name="xpr", bufs=XPR_BUFS) as xpr, \
             tc.tile_pool(name="p1x", bufs=3) as p1x, \
             tc.tile_pool(name="p1ps", bufs=2, space="PSUM") as p1ps, \
             tc.tile_pool(name="st", bufs=2) as st, \
             tc.tile_pool(name="ch", bufs=2) as ch, \
             tc.tile_pool(name="gps_fo", bufs=2, space="PSUM") as gps_fo, \
             tc.tile_pool(name="gps_gi", bufs=2, space="PSUM") as gps_gi, \
             tc.tile_pool(name="tps", bufs=1, space="PSUM") as tps, \
             tc.tile_pool(name="dps", bufs=1, space="PSUM") as dps:

            # ---- persistent weights / identity ----
            Wx_t, Wh_t = [], []
            for k in range(4):
                wt = wpool.tile([128, NG], BF, tag=f"wx{k}", name=f"wx{k}")
                nc.sync.dma_start(wt[:], Wx[128 * k:128 * (k + 1), :])
                Wx_t.append(wt)
            for k in range(4):
                wt = wpool.tile([128, NG], BF, tag=f"wh{k}", name=f"wh{k}")
                nc.sync.dma_start(wt[:], Wh[128 * k:128 * (k + 1), :])
                Wh_t.append(wt)
            bias_t = wpool.tile([1, NG], BF)
            nc.sync.dma_start(bias_t[:], bias[:, :])
            ones_t = cpool.tile([1, 128], BF)
            nc.vector.memset(ones_t[:, :], 1.0)
            ident = cpool.tile([128, 128], BF)
            make_identity(nc, ident[:, :])

            # ---- gate psum ring buffers, kept by step parity ----
            P_fo, P_gi = [None, None], [None, None]
            for z in range(2):
                P_fo[z] = gps_fo.tile([128, 256], FP, tag="Pfo", name=f"Pfo{z}")
                nc.vector.memset(P_fo[z][:, :], 0.0)
                P_gi[z] = gps_gi.tile([128, 256], FP, tag="Pgi", name=f"Pgi{z}")
                nc.vector.memset(P_gi[z][:, :], 0.0)

            # ---- x-part chunklet machinery (folded phase 1) ----
            xp_tiles = {}       # m -> 4D tile [128 tok, 4 G, 4 hc, 128]
            state = {"c": 0, "xm": None}

            def emit_chunklet():
                c = state["c"]
                if c >= 4 * NM:
                    return
                state["c"] += 1
                m, n = divmod(c, 4)
                if n == 0:
                    xm = p1x.tile([128, 4, 128], BF, tag="xm", name="xm")
                    for k in range(4):
                        nc.sync.dma_start(
                            xm[:, k, :],
                            xT[128 * k:128 * (k + 1), 128 * m:128 * (m + 1)])
                    xp_tiles[m] = xpr.tile(
                        [128, 4, 4, 128], BF, tag="xpr", name=f"xp{m}")
                    state["xm"] = xm
                xm = state["xm"]
                ps = p1ps.tile([128, 512], FP, tag="ps1", name="ps1")
                for hf in range(2):
                    sl = slice(512 * n + 256 * hf, 512 * n + 256 * (hf + 1))
                    for k in range(4):
                        nc.tensor.matmul(
                            ps[:, 256 * hf:256 * (hf + 1)], xm[:, k, :],
                            Wx_t[k][:, sl], start=(k == 0), stop=False)
                    nc.tensor.matmul(
                        ps[:, 256 * hf:256 * (hf + 1)], ones_t[:, :],
                        bias_t[:, sl], start=False, stop=True)
                # psum -> sbuf bf16, split + alternating engines to bound
                # the preemption delay on the tail's ACT/DVE chain
                for hf in range(2):
                    d2 = xp_tiles[m][:, n, 2 * hf:2 * (hf + 1), :]
                    s2 = ps[:, 256 * hf:256 * (hf + 1)]
                    if (c + hf) % 2 == 0:
                        nc.scalar.copy(d2, s2)
                    else:
                        nc.vector.tensor_copy(d2, s2)

            def emit_inject(t):
                """x-part injection for step t.  start=True clears the whole
                32-partition column group's has_written bits, so exactly ONE
                start mm per (bank, column-group)."""
                m, r = divmod(t, 8)
                w0 = 32 * (r // 2)
                so = B * (r % 2)
                xp4 = xp_tiles[m]
                for gl, Pb in ((0, P_gi[t % 2]), (2, P_fo[t % 2])):
                    for hc in range(4):
                        nc.tensor.matmul(
                            Pb[32 * hc:32 * hc + B, 0:256],
                            ident[w0:w0 + 32, w0 + so:w0 + so + B],
                            xp4[w0:w0 + 32, gl:gl + 2, hc, :],
                            start=True, stop=False,
                            tile_position=(w0, 32 * hc))

            # HAM warm-keeper target bank (values never read)
            dummy_ps = dps.tile([128, 256], FP, tag="dummy", name="dummy")

            # ---- zero-init state ----
            c_prev = st.tile([112, 128], FP, tag="c", name="c0")
            nc.vector.memset(c_prev[:, :], 0.0)
            hTs_prev = st.tile([128, 112], BF, tag="hTs", name="hTs0")
            nc.vector.memset(hTs_prev[:, :], 0.0)

            # ---- prologue: first chunklets + inject(0) ----
            for _ in range(N_PRO):
                emit_chunklet()
            emit_inject(0)

            for t in range(L):
                Pfo = P_fo[t % 2]
                Pgi = P_gi[t % 2]

                # h-part matmuls: bank gi first (its tail chain is longest)
                for gl, Pb in ((0, Pgi), (2, Pfo)):
                    for k in range(4):
                        for Gb in range(2):
                            for hc in range(4):
                                nc.tensor.matmul(
                                    Pb[32 * hc:32 * hc + B,
                                       128 * Gb:128 * (Gb + 1)],
                                    hTs_prev[:, 32 * k:32 * k + B],
                                    Wh_t[k][:, ((gl + Gb) * 4 + hc) * 128:
                                            ((gl + Gb) * 4 + hc + 1) * 128],
                                    start=False, stop=(k == 3),
                                    tile_position=(0, 32 * hc))

                # PE filler at LOW priority: the ready-heap scheduler runs
                # it only when nothing critical is ready, and tail semaphore
                # thresholds exclude it.
                with tc.high_priority(offset=-1_000_000):
                    if t + 1 < L:
                        emit_inject(t + 1)
                    if t % 2 == 0:
                        emit_chunklet()
                with tc.high_priority(offset=-100_000_000):
                    for dz in range(6):
                        nc.tensor.matmul(
                            dummy_ps[0:B, 0:256], ident[0:128, 0:B],
                            Wx_t[0][:, 0:256], start=True, stop=True)

                # ---- tail (all-sigmoid; see header) ----
                # bank gi: one merged sigmoid over [2g|i]; fp32 so the
                # (sigma(2g) - 0.5) subtraction does not cancel in bf16
                s_gi = ch.tile([112, 256], FP, tag="sgi", name="sgi")
                nc.scalar.activation(s_gi[:, :], Pgi[0:112, :], AF.Sigmoid)
                # t2h = (sig(2g) - 0.5) * sig(i)  ==  tanh(g)*sig(i)/2
                t2h = ch.tile([112, 128], FP, tag="t2h", name="t2h")
                nc.vector.scalar_tensor_tensor(
                    t2h[:, :], s_gi[:, 0:128], 0.5, s_gi[:, 128:256],
                    AluOpType.subtract, AluOpType.mult)
                # bank fo: one merged sigmoid over [f|o]
                s_fo = ch.tile([112, 256], BF, tag="sfo", name="sfo")
                nc.scalar.activation(s_fo[:, :], Pfo[0:112, :], AF.Sigmoid)
                t1 = ch.tile([112, 128], FP, tag="t1", name="t1")
                nc.vector.tensor_mul(t1[:, :], s_fo[:, 0:128], c_prev[:, :])
                # c_new = 2*t2h + t1   (t2h = tanh(g)*sig(i)/2)
                c_new = st.tile([112, 128], FP, tag="c", name="c")
                nc.vector.scalar_tensor_tensor(
                    c_new[:, :], t2h[:, :], 2.0, t1[:, :],
                    AluOpType.mult, AluOpType.add)
                th = ch.tile([112, 128], BF, tag="th", name="th")
                nc.scalar.activation(th[:, :], c_new[:, :], AF.Tanh)

                # transposes (PE) + one fused DVE op -> h/2 transposed.
                # DVE reads at most one PSUM operand, so T(sigma_o) is staged
                # to SBUF early (off the critical chain).
                TT = tps.tile([128, 224], BF, tag="TT", name="TT")
                nc.tensor.transpose(
                    TT[:, 0:112], s_fo[0:112, 128:256], ident[0:112, 0:112])
                To_sb = ch.tile([128, 112], BF, tag="To", name="To")
                nc.vector.tensor_copy(To_sb[:, :], TT[:, 0:112])
                nc.tensor.transpose(
                    TT[:, 112:224], th[0:112, :], ident[0:112, 0:112])
                # h = tanh(c)^T * sig(o)^T
                hTs_new = st.tile([128, 112], BF, tag="hTs", name="hTs")
                nc.vector.tensor_mul(
                    hTs_new[:, :], TT[:, 112:224], To_sb[:, :])

                nc.sync.dma_start(out_h[t, :, :], hTs_new[:, :])

                c_prev = c_new
                hTs_prev = hTs_new
    nc.compile()
    return nc


def _host_prepare(x_full, weights, direction, bslice):
    xs = x_full[bslice]
    if direction == "bw":
        xs = xs[:, ::-1, :]
    xT = np.ascontiguousarray(xs.transpose(2, 1, 0).reshape(D, TOK))
    Wc = np.concatenate(
        [np.asarray(weights[f"W_{direction}_{n}"]).T for n in GATES], axis=1)
    bc = np.concatenate(
        [np.asarray(weights[f"b_{direction}_{n}"]) for n in GATES])[None, :]
    # tanh-as-sigmoid folding: gate g cols x2 (block holds 2g) -- exact
    # power-of-2 scaling in bf16.
    Wc = Wc.copy(); bc = bc.copy()
    Wc[:, 0:H] *= 2.0
    bc[:, 0:H] *= 2.0
    bf = ml_dtypes.bfloat16
    return {"xT": xT.astype(bf),
            "Wx": np.ascontiguousarray(Wc[:D]).astype(bf),
            "Wh": np.ascontiguousarray(Wc[D:]).astype(bf),
            "bias": np.ascontiguousarray(bc).astype(bf)}


def prepare(inputs):
    """Build (cached) the bass program and the 8 per-core input maps."""
    global _CACHED_NC
    inputs = {k: np.asarray(v) for k, v in inputs.items()}
    x = inputs["x"]
    Bx, Lx, _ = x.shape
    assert (Bx, Lx) == (64, L)

    if _CACHED_NC is None:
        _CACHED_NC = _build()
    nc = _CACHED_NC

    in_maps = []
    for ci in range(8):
        d = "fw" if ci < 4 else "bw"
        bs = (ci % 4) * B
        in_maps.append(_host_prepare(x, inputs, d, slice(bs, bs + B)))
    return nc, in_maps


def _unshard_core(oh):
    """out_h [L, 128, 112] bf16 (transposed h) -> [L, 16, 512] f32.
    h[t, b, 128*k + hl] = oh[t, hl, 32*k + b]."""
    a = np.asarray(oh).astype(np.float32)
    parts = [a[:, :, 32 * k:32 * k + B].transpose(0, 2, 1) for k in range(4)]
    return np.concatenate(parts, axis=2)  # [L, B, 512]


def kernel(**inputs):
    inputs = {k: np.asarray(v) for k, v in inputs.items()}
    x = inputs["x"]
    Bx = x.shape[0]
    nc, in_maps = prepare(inputs)
    meta = [("fw" if ci < 4 else "bw", (ci % 4) * B) for ci in range(8)]

    res = run_bass_kernel_spmd(nc, in_maps, core_ids=list(range(8)))

    hf = np.zeros((L, Bx, H), np.float32)
    hb = np.zeros((L, Bx, H), np.float32)
    for ci in range(8):
        d, bs = meta[ci]
        oh = _unshard_core(res.results[ci]["out_h"])  # (L, 16, H) time-major
        if d == "fw":
            hf[:, bs:bs + B, :] = oh
        else:
            hb[:, bs:bs + B, :] = oh[::-1]

    # faithful to the reference: stack time-major, flatten, hstack, reshape
    flat = np.concatenate([hf.reshape(-1, H), hb.reshape(-1, H)], axis=1)
    return flat.reshape(Bx, L, 2 * H).astype(np.float32)


# revision 16
# speedup vs baseline: 1.3223x; 1.2589x over previous
"""BiLSTM (B=64, L=256, D=512, H=512) on 8 Trainium2 NeuronCores.

Sharding: 8 cores = 2 directions x 4 batch-slices of 16 (weights replicated
per direction, sequential time loop local to each core).  Backward-direction
cores receive time-reversed x, so every core runs the identical SPMD program.

v6 design:
  - all matmuls bf16 (1 cycle/moving-row, single pass).
  - gate layout: TWO psum banks per step, Pfo=[f,o] and Pgi=[g,i], each
    [128, 256] with gate blocks of 128 cols; block = [4 h-chunks x
    32-partition groups, 128 free] (partition = 32*hc + b, free = h%128).
    Two banks -> the ACT tail for [f,o] starts while [g,i] matmuls run.
  - x-part GEMM is FOLDED into the recurrence loop (one (m,n) chunklet
    every 2 steps + 16-chunklet prologue): keeps the PE busy enough that
    the HAM clock gate stays at 2.4 GHz (v2 ran at 1.2 GHz 88% of the
    time), and hides the whole phase-1 cost in tail-wait PE idle slots.
  - transpose factoring: hT = T(sigma_o) * T(tanh c) elementwise; the two
    PE transposes run inside the tail and ONE DVE mul yields the bf16
    stationary for the next step directly.  out_h is written in the
    transposed layout [hl, 32k+b]; the host unscrambles.
  - the Tile scheduler is a per-engine ready-heap ordered by emission
    priority: all filler PE work (inject(t+1), x-part chunklets, HAM
    warm-keeper dummies) is emitted at LOW priority so the tail's
    semaphore thresholds never include it, and it soaks up PE idle.
  - low-priority dummy matmuls keep the PE busy so the HAM clock gate
    ramps to 2.4 GHz and stays there (otherwise every matmul runs at
    1.2 GHz: bursts are too short for the 3.4 us activity window).
  - merged-activation tail: gate g's weights are doubled (exact in bf16)
    so its block holds 2g, and tanh(g) = 2*sigmoid(2g)-1 lets ONE sigmoid
    cover the whole [2g|i] bank (s_gi kept fp32 to avoid the sigma-0.5
    cancellation); fused scalar_tensor_tensor ops recover the cell
    update.  3 ACT ops/step (sigma_gi, sigma_fo, tanh_c) instead of 5.
"""

import numpy as np
import ml_dtypes

from concourse import tile, mybir, bacc
from concourse.bass_utils import run_bass_kernel_spmd
from concourse.masks import make_identity
from concourse.alu_op_type import AluOpType

FP = mybir.dt.float32
BF = mybir.dt.bfloat16
AF = mybir.ActivationFunctionType

B = 16        # local batch per core
L = 256       # timesteps
D = 512       # input dim
H = 512       # hidden
NG = 4 * H    # gate width
TOK = L * B   # tokens per core
NM = TOK // 128  # 32 x-part token tiles

# gate order on device: blocks [g, i, f, o]; banks: gi = blocks 0:2,
# fo = blocks 2:4
GATES = "gifo"

N_PRO = 16    # prologue chunklets (4 m-tiles)
XPR_BUFS = 6  # xp ring depth in m-tiles

_CACHED_NC = None


def _build():
    nc = bacc.Bacc("TRN2", target_bir_lowering=False, debug=False)

    xT = nc.dram_tensor("xT", [D, TOK], BF, kind="ExternalInput").ap()
    Wx = nc.dram_tensor("Wx", [D, NG], BF, kind="ExternalInput").ap()
    Wh = nc.dram_tensor("Wh", [H, NG], BF, kind="ExternalInput").ap()
    bias = nc.dram_tensor("bias", [1, NG], BF, kind="ExternalInput").ap()
    out_h = nc.dram_tensor("out_h", [L, 128, 112], BF, kind="ExternalOutput").ap()

    with tile.TileContext(nc, trace_sim=False) as tc:
        with tc.tile_pool(name="wpool", bufs=1) as wpool, \
             tc.tile_pool(name="cpool", bufs=1) as cpool, \
             tc.tile_pool(name="xpr", bufs=XPR_BUFS) as xpr, \
             tc.tile_pool(name="p1x", bufs=3) as p1x, \
             tc.tile_pool(name="p1ps", bufs=2, space="PSUM") as p1ps, \
             tc.tile_pool(name="st", bufs=2) as st, \
             tc.tile_pool(name="ch", bufs=2) as ch, \
             tc.tile_pool(name="gps_fo", bufs=2, space="PSUM") as gps_fo, \
             tc.tile_pool(name="gps_gi", bufs=2, space="PSUM") as gps_gi, \
             tc.tile_pool(name="tps", bufs=1, space="PSUM") as tps, \
             tc.tile_pool(name="dps", bufs=1, space="PSUM") as dps:

            # ---- persistent weights / identity ----
            Wx_t, Wh_t = [], []
            for k in range(4):
                wt = wpool.tile([128, NG], BF, tag=f"wx{k}", name=f"wx{k}")
                nc.sync.dma_start(wt[:], Wx[128 * k:128 * (k + 1), :])
                Wx_t.append(wt)
            for k in range(4):
                wt = wpool.tile([128, 4, 4, 128], BF, tag=f"wh{k}", name=f"wh{k}")
                nc.sync.dma_start(wt[:, :, :, :], Wh[128 * k:128 * (k + 1), :])
                Wh_t.append(wt)
            bias_t = wpool.tile([1, NG], BF)
            nc.sync.dma_start(bias_t[:], bias[:, :])
            ones_t = cpool.tile([1, 128], BF)
            nc.vector.memset(ones_t[:, :], 1.0)
            ident = cpool.tile([128, 128], BF)
            make_identity(nc, ident[:, :])

            # ---- gate psum ring buffers, kept by step parity ----
            P_fo, P_gi = [None, None], [None, None]
            for z in range(2):
                P_fo[z] = gps_fo.tile([128, 256], FP, tag="Pfo", name=f"Pfo{z}")
                nc.vector.memset(P_fo[z][:, :], 0.0)
                P_gi[z] = gps_gi.tile([128, 256], FP, tag="Pgi", name=f"Pgi{z}")
                nc.vector.memset(P_gi[z][:, :], 0.0)

            # ---- x-part chunklet machinery (folded phase 1) ----
            xp_tiles = {}       # m -> 4D tile [128 tok, 4 G, 4 hc, 128]
            state = {"c": 0, "xm": None}

            def emit_chunklet():
                c = state["c"]
                if c >= 4 * NM:
                    return
                state["c"] += 1
                m, n = divmod(c, 4)
                if n == 0:
                    xm = p1x.tile([128, 4, 128], BF, tag="xm", name="xm")
                    for k in range(4):
                        nc.sync.dma_start(
                            xm[:, k, :],
                            xT[128 * k:128 * (k + 1), 128 * m:128 * (m + 1)])
                    xp_tiles[m] = xpr.tile(
                        [128, 4, 4, 128], BF, tag="xpr", name=f"xp{m}")
                    state["xm"] = xm
                xm = state["xm"]
                ps = p1ps.tile([128, 512], FP, tag="ps1", name="ps1")
                for hf in range(2):
                    sl = slice(512 * n + 256 * hf, 512 * n + 256 * (hf + 1))
                    for k in range(4):
                        nc.tensor.matmul(
                            ps[:, 256 * hf:256 * (hf + 1)], xm[:, k, :],
                            Wx_t[k][:, sl], start=(k == 0), stop=False)
                    nc.tensor.matmul(
                        ps[:, 256 * hf:256 * (hf + 1)], ones_t[:, :],
                        bias_t[:, sl], start=False, stop=True)
                # psum -> sbuf bf16, split + alternating engines to bound
                # the preemption delay on the tail's ACT/DVE chain
                for hf in range(2):
                    d2 = xp_tiles[m][:, n, 2 * hf:2 * (hf + 1), :]
                    s2 = ps[:, 256 * hf:256 * (hf + 1)]
                    if (c + hf) % 2 == 0:
                        nc.scalar.copy(d2, s2)
                    else:
                        nc.vector.tensor_copy(d2, s2)

            def emit_inject(t):
                """x-part injection for step t.  start=True clears the whole
                32-partition column group's has_written bits, so exactly ONE
                start mm per (bank, column-group)."""
                m, r = divmod(t, 8)
                w0 = 32 * (r // 2)
                so = B * (r % 2)
                xp4 = xp_tiles[m]
                for gl, Pb in ((0, P_gi[t % 2]), (2, P_fo[t % 2])):
                    for hc in range(4):
                        nc.tensor.matmul(
                            Pb[32 * hc:32 * hc + B, 0:256],
                            ident[w0:w0 + 32, w0 + so:w0 + so + B],
                            xp4[w0:w0 + 32, gl:gl + 2, hc, :],
                            start=True, stop=False,
                            tile_position=(w0, 32 * hc))

            # HAM warm-keeper target bank (values never read)
            dummy_ps = dps.tile([128, 512], FP, tag="dummy", name="dummy")

            # ---- zero-init state ----
            c_prev = st.tile([112, 128], FP, tag="c", name="c0")
            nc.vector.memset(c_prev[:, :], 0.0)
            hTs_prev = st.tile([128, 112], BF, tag="hTs", name="hTs0")
            nc.vector.memset(hTs_prev[:, :], 0.0)

            # ---- prologue: first chunklets + inject(0) ----
            for _ in range(N_PRO):
                emit_chunklet()
            emit_inject(0)

            for t in range(L):
                Pfo = P_fo[t % 2]
                Pgi = P_gi[t % 2]

                # h-part matmuls: bank gi first (its tail chain is longest).
                # One N=256 mm per (bank, k, hc): the moving AP gathers the
                # bank's two gate blocks (stride 512) -> half the PE
                # instruction count (the PE sequencer feeds ~2 inst/34ns).
                for gl, Pb in ((0, Pgi), (2, Pfo)):
                    for k in range(4):
                        for hc in range(4):
                            nc.tensor.matmul(
                                Pb[32 * hc:32 * hc + B, 0:256],
                                hTs_prev[:, 32 * k:32 * k + B],
                                Wh_t[k][:, gl:gl + 2, hc, :],
                                start=False, stop=(k == 3),
                                tile_position=(0, 32 * hc))

                # PE filler at LOW priority: the ready-heap scheduler runs
                # it only when nothing critical is ready, and tail semaphore
                # thresholds exclude it.
                with tc.high_priority(offset=-1_000_000):
                    if t + 1 < L:
                        emit_inject(t + 1)
                    if t % 2 == 0:
                        emit_chunklet()
                with tc.high_priority(offset=-100_000_000):
                    for dz in range(4):
                        nc.tensor.matmul(
                            dummy_ps[0:B, 0:512], ident[0:128, 0:B],
                            Wx_t[0][:, 0:512], start=True, stop=True)

                # ---- tail (all-sigmoid; see header) ----
                # bank gi: one merged sigmoid over [2g|i]; fp32 so the
                # (sigma(2g) - 0.5) subtraction does not cancel in bf16
                s_gi = ch.tile([112, 256], FP, tag="sgi", name="sgi")
                nc.scalar.activation(s_gi[:, :], Pgi[0:112, :], AF.Sigmoid)
                # t2h = (sig(2g) - 0.5) * sig(i)  ==  tanh(g)*sig(i)/2
                t2h = ch.tile([112, 128], FP, tag="t2h", name="t2h")
                nc.vector.scalar_tensor_tensor(
                    t2h[:, :], s_gi[:, 0:128], 0.5, s_gi[:, 128:256],
                    AluOpType.subtract, AluOpType.mult)
                # bank fo: one merged sigmoid over [f|o]
                s_fo = ch.tile([112, 256], BF, tag="sfo", name="sfo")
                nc.scalar.activation(s_fo[:, :], Pfo[0:112, :], AF.Sigmoid)
                t1 = ch.tile([112, 128], FP, tag="t1", name="t1")
                nc.vector.tensor_mul(t1[:, :], s_fo[:, 0:128], c_prev[:, :])
                # c_new = 2*t2h + t1   (t2h = tanh(g)*sig(i)/2)
                c_new = st.tile([112, 128], FP, tag="c", name="c")
                nc.vector.scalar_tensor_tensor(
                    c_new[:, :], t2h[:, :], 2.0, t1[:, :],
                    AluOpType.mult, AluOpType.add)
                th = ch.tile([112, 128], BF, tag="th", name="th")
                nc.scalar.activation(th[:, :], c_new[:, :], AF.Tanh)

                # transposes (PE) + one fused DVE op -> h/2 transposed.
                # DVE reads at most one PSUM operand, so T(sigma_o) is staged
                # to SBUF early (off the critical chain).
                TT = tps.tile([128, 224], BF, tag="TT", name="TT")
                nc.tensor.transpose(
                    TT[:, 0:112], s_fo[0:112, 128:256], ident[0:112, 0:112])
                To_sb = ch.tile([128, 112], BF, tag="To", name="To")
                nc.vector.tensor_copy(To_sb[:, :], TT[:, 0:112])
                nc.tensor.transpose(
                    TT[:, 112:224], th[0:112, :], ident[0:112, 0:112])
                # h = tanh(c)^T * sig(o)^T; split so k=0,1 chunks release
                # the next step's first matmuls early
                hTs_new = st.tile([128, 112], BF, tag="hTs", name="hTs")
                nc.vector.tensor_mul(
                    hTs_new[:, 0:64], TT[:, 112:176], To_sb[:, 0:64])
                nc.vector.tensor_mul(
                    hTs_new[:, 64:112], TT[:, 176:224], To_sb[:, 64:112])

                nc.sync.dma_start(out_h[t, :, :], hTs_new[:, :])

                c_prev = c_new
                hTs_prev = hTs_new
    nc.compile()
    return nc


def _host_prepare(x_full, weights, direction, bslice):
    xs = x_full[bslice]
    if direction == "bw":
        xs = xs[:, ::-1, :]
    xT = np.ascontiguousarray(xs.transpose(2, 1, 0).reshape(D, TOK))
    Wc = np.concatenate(
        [np.asarray(weights[f"W_{direction}_{n}"]).T for n in GATES], axis=1)
    bc = np.concatenate(
        [np.asarray(weights[f"b_{direction}_{n}"]) for n in GATES])[None, :]
    # tanh-as-sigmoid folding: gate g cols x2 (block holds 2g) -- exact
    # power-of-2 scaling in bf16.
    Wc = Wc.copy(); bc = bc.copy()
    Wc[:, 0:H] *= 2.0
    bc[:, 0:H] *= 2.0
    bf = ml_dtypes.bfloat16
    return {"xT": xT.astype(bf),
            "Wx": np.ascontiguousarray(Wc[:D]).astype(bf),
            "Wh": np.ascontiguousarray(Wc[D:]).astype(bf),
            "bias": np.ascontiguousarray(bc).astype(bf)}


def prepare(inputs):
    """Build (cached) the bass program and the 8 per-core input maps."""
    global _CACHED_NC
    inputs = {k: np.asarray(v) for k, v in inputs.items()}
    x = inputs["x"]
    Bx, Lx, _ = x.shape
    assert (Bx, Lx) == (64, L)

    if _CACHED_NC is None:
        _CACHED_NC = _build()
    nc = _CACHED_NC

    in_maps = []
    for ci in range(8):
        d = "fw" if ci < 4 else "bw"
        bs = (ci % 4) * B
        in_maps.append(_host_prepare(x, inputs, d, slice(bs, bs + B)))
    return nc, in_maps


def _unshard_core(oh):
    """out_h [L, 128, 112] bf16 (transposed h) -> [L, 16, 512] f32.
    h[t, b, 128*k + hl] = oh[t, hl, 32*k + b]."""
    a = np.asarray(oh).astype(np.float32)
    parts = [a[:, :, 32 * k:32 * k + B].transpose(0, 2, 1) for k in range(4)]
    return np.concatenate(parts, axis=2)  # [L, B, 512]


def kernel(**inputs):
    inputs = {k: np.asarray(v) for k, v in inputs.items()}
    x = inputs["x"]
    Bx = x.shape[0]
    nc, in_maps = prepare(inputs)
    meta = [("fw" if ci < 4 else "bw", (ci % 4) * B) for ci in range(8)]

    res = run_bass_kernel_spmd(nc, in_maps, core_ids=list(range(8)))

    hf = np.zeros((L, Bx, H), np.float32)
    hb = np.zeros((L, Bx, H), np.float32)
    for ci in range(8):
        d, bs = meta[ci]
        oh = _unshard_core(res.results[ci]["out_h"])  # (L, 16, H) time-major
        if d == "fw":
            hf[:, bs:bs + B, :] = oh
        else:
            hb[:, bs:bs + B, :] = oh[::-1]

    # faithful to the reference: stack time-major, flatten, hstack, reshape
    flat = np.concatenate([hf.reshape(-1, H), hb.reshape(-1, H)], axis=1)
    return flat.reshape(Bx, L, 2 * H).astype(np.float32)
